# revision 11
# baseline (speedup 1.0000x reference)
"""Chamfer loss kernel for Trainium2 (8 NeuronCores, SPMD).

Problem: trgt [8,4096,3], pred [8,4096,3] fp32 ->
  (accuracy, complete, chamfer) scalars, where per batch b:
    d2[n,m] = ||t_n - p_m||^2
    complete_b = mean_n sqrt(min_m d2)   (target -> pred)
    accuracy_b = mean_m sqrt(min_n d2)   (pred -> target)

Strategy (one batch per core, data-parallel over b):
  * Host prep: d2 = t2 + p2 - 2 t.p as an augmented K=13 bf16 matmul
    (hi/lo bf16 split keeps ~fp32 precision; PSUM accumulates fp32).
  * m is split into 4 groups of 1024: groups 0-2 use a SOFTMIN drain
    (ACT computes E=exp(-beta*(d2-C)) bf16 + per-row accumulated sums;
    PE ones-matmuls accumulate column sums Sigma_n E in PSUM across all
    32 n-chunks), group 3 uses exact hard-min on DVE (ttr row-min +
    bf16 colacc).  min_n d2 = C - ln(S)/beta recovered on host.
  * This splits the PSUM-drain work between ACT (softmin groups, its
    exp pass replaces the cast-copy) and DVE (hard group) while the PE
    absorbs the column reduction; no PE-transpose tail, no min-tree.
  * Device outputs raw S_row/S_col/rowmin/colacc; host does the tiny
    log/sqrt/mean finish in fp64.
"""

import numpy as np
import ml_dtypes

B, N, M, P = 8, 4096, 4096, 128
NI = N // P          # 32 n-chunks
QW = 1024            # PSUM unit width (2 banks)
NG = M // QW         # 4 m-groups
NSOFT = 3            # groups 0..2 soft, group 3 hard
KROWS = 13
N_CORES = 8

BETA = 600.0
CSTAB = 0.0
BIG = 3.0e38
SCLAMP = 1e-37
S_FALLBACK = 1e-30   # sums below this -> host recomputes that row/col exactly
HARD_LO = NSOFT * QW  # 3072

_CACHE = {}


def _build_program():
    from contextlib import ExitStack
    import concourse.tile as tile
    from concourse import bacc, mybir

    f32 = mybir.dt.float32
    bf16 = mybir.dt.bfloat16
    mn = mybir.AluOpType.min
    X = mybir.AxisListType.X
    EXP = mybir.ActivationFunctionType.Exp

    nc = bacc.Bacc("TRN2", target_bir_lowering=False, debug=False,
                   num_devices=N_CORES)
    lhs_d = nc.dram_tensor("lhs", [P, N], bf16, kind="ExternalInput").ap()
    rhs_d = nc.dram_tensor("rhs", [P, M], bf16, kind="ExternalInput").ap()
    srow_d = nc.dram_tensor("srow", [P, NSOFT * NI], f32,
                            kind="ExternalOutput").ap()
    rowh_d = nc.dram_tensor("rowh", [P, NI], f32, kind="ExternalOutput").ap()
    colh_d = nc.dram_tensor("colh", [P, QW], bf16, kind="ExternalOutput").ap()
    scol_d = nc.dram_tensor("scol", [P, QW], f32, kind="ExternalOutput").ap()

    with tile.TileContext(nc) as tc:
        with ExitStack() as ctx:
            consts = ctx.enter_context(tc.tile_pool(name="consts", bufs=1))
            epool = ctx.enter_context(tc.tile_pool(name="epool", bufs=3))
            sqp = ctx.enter_context(tc.tile_pool(name="sqp", bufs=2))

            lhs_sb = consts.tile([P, N], bf16)
            rhs_sb = consts.tile([P, M], bf16)
            for c in range(4):
                nc.sync.dma_start(lhs_sb[:, c * 1024:(c + 1) * 1024],
                                  lhs_d[:, c * 1024:(c + 1) * 1024])
                nc.sync.dma_start(rhs_sb[:, c * 1024:(c + 1) * 1024],
                                  rhs_d[:, c * 1024:(c + 1) * 1024])
            ones_sb = consts.tile([P, 1], bf16)
            nc.vector.memset(ones_sb, 1.0)
            bigt = consts.tile([P, QW], bf16)
            nc.vector.memset(bigt, BIG)
            colacc = consts.tile([P, QW], bf16)
            nc.vector.memset(colacc, BIG)
            srow_sb = consts.tile([P, NSOFT * NI], f32)
            rowh_sb = consts.tile([P, NI], f32)
            scol_sb = consts.tile([P, QW], f32)

            with tc.tile_pool(name="psq", bufs=3, space="PSUM") as psq, \
                 tc.tile_pool(name="psacc", bufs=1, space="PSUM") as psacc:
                acc = psacc.tile([P, QW], f32)
                prev_E = None   # (chunk, [E-tiles for groups 0..2])
                for i in range(NI):
                    # ---- d2 matmuls: 8 m-slices of 512, row-band packed
                    quads = []
                    for g in range(NG):
                        quad = psq.tile([P, QW], f32, tag="quad")
                        for h in range(2):
                            sl = 2 * g + h          # m-slice index 0..7
                            r = sl % 4              # row band
                            mlo = sl * 512
                            nc.tensor.matmul(
                                quad[:, h * 512:(h + 1) * 512],
                                lhs_sb[32 * r:32 * r + KROWS,
                                       i * P:(i + 1) * P],
                                rhs_sb[32 * r:32 * r + KROWS,
                                       mlo:mlo + 512],
                                start=True, stop=True,
                                tile_position=(32 * r, 0),
                            )
                        quads.append(quad)

                    # ---- ACT drains for soft groups: E=exp(-b(d2-C)), and
                    # per-row partial sums into srow_sb
                    etiles = []
                    for g in range(NSOFT):
                        E = epool.tile([P, QW], bf16, tag=f"E{g}")
                        nc.scalar.activation(
                            E, quads[g], EXP,
                            bias=BETA * CSTAB, scale=-BETA,
                            accum_out=srow_sb[:, g * NI + i:g * NI + i + 1],
                        )
                        etiles.append(E)

                    # ---- ones-matmuls for the PREVIOUS chunk's E tiles
                    # (keeps PE from stalling on this chunk's ACT drains)
                    if prev_E is not None:
                        pi, pE = prev_E
                        for g in range(NSOFT):
                            for h in range(2):
                                nc.tensor.matmul(
                                    acc[32 * g:32 * g + 1,
                                        h * 512:(h + 1) * 512],
                                    ones_sb, pE[g][:, h * 512:(h + 1) * 512],
                                    start=(pi == 0), stop=(pi == NI - 1),
                                    tile_position=(0, 32 * g),
                                )
                    prev_E = (i, etiles)

                    # ---- hard group: DVE exit (bf16 copy), row-min via a
                    # short tree, and colacc merge
                    sq = sqp.tile([P, QW], bf16, tag="sq")
                    nc.vector.tensor_tensor(sq, quads[NSOFT], bigt, mn)
                    t1 = sqp.tile([P, QW // 2], bf16, tag="t1")
                    nc.vector.tensor_tensor(
                        t1, sq[:, :QW // 2], sq[:, QW // 2:], mn)
                    nc.vector.tensor_reduce(
                        rowh_sb[:, i:i + 1], t1, X, mn)
                    nc.vector.tensor_tensor(colacc, colacc, sq, mn)

                # last chunk's ones-matmuls
                pi, pE = prev_E
                for g in range(NSOFT):
                    for h in range(2):
                        nc.tensor.matmul(
                            acc[32 * g:32 * g + 1, h * 512:(h + 1) * 512],
                            ones_sb, pE[g][:, h * 512:(h + 1) * 512],
                            start=(pi == 0), stop=(pi == NI - 1),
                            tile_position=(0, 32 * g),
                        )

                # drain acc to SBUF (rows 0/32/64 valid), DMA everything out
                nc.vector.tensor_copy(scol_sb, acc)
                nc.sync.dma_start(scol_d, scol_sb)
                nc.sync.dma_start(srow_d, srow_sb)
                nc.sync.dma_start(rowh_d, rowh_sb)
                nc.sync.dma_start(colh_d, colacc)

    nc.compile()
    return nc


def _host_prep(trgt, pred):
    """Per-batch augmented bf16 hi/lo matrices, 4x replicated on partitions.

    d2[n,m] = sum_k lhs[k,n]*rhs[k,m] with rows:
      k0-2 : th_d      x -2 ph_d
      k3-5 : th_d      x -2 pl_d
      k6-8 : tl_d      x -2 ph_d
      k9,10: t2h, t2l  x  1
      k11,12: 1        x  p2h, p2l
    """
    bf = ml_dtypes.bfloat16
    in_maps = []
    for b in range(B):
        t = np.asarray(trgt[b], dtype=np.float64)
        p = np.asarray(pred[b], dtype=np.float64)
        th = t.astype(bf).astype(np.float64)
        tl = (t - th).astype(bf).astype(np.float64)
        ph = p.astype(bf).astype(np.float64)
        pl = (p - ph).astype(bf).astype(np.float64)
        t2 = (t * t).sum(-1)
        p2 = (p * p).sum(-1)
        t2h = t2.astype(bf).astype(np.float64)
        t2l = (t2 - t2h).astype(bf).astype(np.float64)
        p2h = p2.astype(bf).astype(np.float64)
        p2l = (p2 - p2h).astype(bf).astype(np.float64)
        on = np.ones(N)
        lhs13 = np.stack([th[:, 0], th[:, 1], th[:, 2],
                          th[:, 0], th[:, 1], th[:, 2],
                          tl[:, 0], tl[:, 1], tl[:, 2],
                          t2h, t2l, on, on])
        rhs13 = np.stack([-2 * ph[:, 0], -2 * ph[:, 1], -2 * ph[:, 2],
                          -2 * pl[:, 0], -2 * pl[:, 1], -2 * pl[:, 2],
                          -2 * ph[:, 0], -2 * ph[:, 1], -2 * ph[:, 2],
                          on, on, p2h, p2l])
        lhs = np.zeros((P, N), dtype=bf)
        rhs = np.zeros((P, M), dtype=bf)
        for r in range(4):
            lhs[32 * r:32 * r + KROWS] = lhs13.astype(bf)
            rhs[32 * r:32 * r + KROWS] = rhs13.astype(bf)
        in_maps.append({"lhs": lhs, "rhs": rhs,
                        "ones": np.ones((P, 1), dtype=bf)})
    return in_maps


def kernel(trgt, pred):
    from concourse.bass_utils import run_bass_kernel_spmd

    trgt = np.asarray(trgt, dtype=np.float32)
    pred = np.asarray(pred, dtype=np.float32)
    assert trgt.shape == (B, N, 3) and pred.shape == (B, M, 3)

    if "nc" not in _CACHE:
        _CACHE["nc"] = _build_program()
    nc = _CACHE["nc"]

    in_maps = _host_prep(trgt, pred)
    res = run_bass_kernel_spmd(nc, in_maps, list(range(N_CORES)))
    comp = np.zeros(B, dtype=np.float64)
    acc = np.zeros(B, dtype=np.float64)
    for b in range(B):
        r = res.results[b]
        srow = np.asarray(r["srow"], dtype=np.float64)   # [128, 96]
        rowh = np.asarray(r["rowh"], dtype=np.float64)   # [128, 32]
        colh = np.asarray(r["colh"], dtype=np.float64)   # [128, 1024]
        scol = np.asarray(r["scol"], dtype=np.float64)   # rows 0/32/64
        t64 = trgt[b].astype(np.float64)                 # [4096, 3]
        p64 = pred[b].astype(np.float64)
        ps = p64[:HARD_LO]                               # soft-region preds
        # complete: per n = min(soft rowmin over m<3072, hard rowmin)
        s_row = (srow[:, 0:NI] + srow[:, NI:2 * NI]
                 + srow[:, 2 * NI:3 * NI])               # [128, 32]
        soft_row = CSTAB - np.log(np.maximum(s_row, SCLAMP)) / BETA
        # n index of (partition p, chunk i) is i*128+p
        fb = s_row < S_FALLBACK
        if fb.any():
            pidx, cidx = np.nonzero(fb)
            nidx = cidx * P + pidx
            tv = t64[nidx]
            d2 = ((tv * tv).sum(-1)[:, None] + (ps * ps).sum(-1)[None, :]
                  - 2.0 * tv @ ps.T)
            soft_row[pidx, cidx] = d2.min(axis=1)
        comp_d2 = np.minimum(soft_row, rowh)             # [128, 32]
        comp[b] = np.sqrt(np.maximum(comp_d2, 0.0)).sum() / N
        # accuracy: soft cols (3*1024 sums) + hard cols (colacc part-min)
        s_col = np.concatenate([scol[0], scol[32], scol[64]])
        soft_col = CSTAB - np.log(np.maximum(s_col, SCLAMP)) / BETA
        fbc = np.nonzero(s_col < S_FALLBACK)[0]
        if fbc.size:
            pv = p64[fbc]
            d2 = ((pv * pv).sum(-1)[:, None] + (t64 * t64).sum(-1)[None, :]
                  - 2.0 * pv @ t64.T)
            soft_col[fbc] = d2.min(axis=1)
        hard_col = colh.min(axis=0)
        acc_d2 = np.concatenate([soft_col, hard_col])
        acc[b] = np.sqrt(np.maximum(acc_d2, 0.0)).sum() / M
    accuracy = np.float32(acc.mean())
    complete = np.float32(comp.mean())
    chamfer = np.float32(0.5 * (accuracy.astype(np.float64)
                                + complete.astype(np.float64)))
    return (accuracy, complete, chamfer)
